# revision 1
# baseline (speedup 1.0000x reference)
"""DYAN encoder (FISTA sparse coding) as a Bass/Tile kernel on 8 trn2 NeuronCores.

Algorithm notes
---------------
reference computes, with D [T=10, K=645] (normalized dictionary), Y = x[0] [10, P]:
    A   = I - D^T D / L,  c = D^T Y / L,  lam = 0.1 / L
    y_0 = x_0 = 0
    for j in 0..99:   (the early-stop never triggers for this data)
        w      = A y_j + c = y_j + (1/L) D^T (Y - D y_j)
        x_{j+1} = softshrink(w, lam)
        y_{j+1} = (1+tt_j) x_{j+1} - tt_j x_j
Since A is I minus a rank-10 term, each iteration only needs thin matmuls:
    u_j = Y - D x_j                    [10, P]   (PE, contraction 645)
    r_j = (1+tt) u_j - tt u_{j-1}      (momentum folded into the residual)
    w   = (1/L) D^T r_j - tt x_{j-1} + (1+tt) x_j + ... (identity parts)
    x_{j+1} = shrink((1/L) D^T r_j - tt x_{j-1}  +  (1+tt) x_j)
The (1/L)(1+tt) / -(1/L)tt scalings ride the PSUM->SBUF copies of u (ScalarE),
the -tt x_{j-1} term is a scaled-identity matmul on PE, and the (1+tt) x_j add
plus softshrink is one fused custom DVE op per chunk.

Sharding: pure data parallel over the pixel dim P (8192 -> 8 x 1024).
"""

import os
import numpy as np

T = 10
NDICT = 161
K = 4 * NDICT + 1          # 645
P_FULL = 8192
N_CORES = 8
P = P_FULL // N_CORES      # 1024
NH = 512                   # psum-bank half width (fp32)
CH = [128, 128, 128, 128, 128, 5]   # K split into partition chunks
OFF = [0, 128, 256, 384, 512, 640]
NITER = 100
LAMBD = 0.1

# debug/ablation flags
ACT_COPY = True      # A/B copies on ScalarE (else VectorE)

_cache = {}


# --------------------------------------------------------------------------- #
# custom DVE ops
# --------------------------------------------------------------------------- #
def _register_dve_op(name, spec):
    import concourse.dve_ops as dve_ops_mod
    from concourse.dve_spec import lower, _has_src1
    from concourse.dve_uop import DveOpSpec

    for o in dve_ops_mod.OPS:
        if o.name == name:
            return o
    row = dve_ops_mod._CUSTOM_DVE_ROW_BASE + len(dve_ops_mod.OPS)
    assert row < 0x20, "DVE opcode rows exhausted"
    shas = {}
    for ver in ("v3", "v4"):
        s = DveOpSpec(name=name, opcode=row, uops=lower(spec, ver=ver),
                      rd1_en=_has_src1(spec))
        shas[ver] = s.sha(ver)
    op = dve_ops_mod.DveOp(name, spec, subdim=False, uops_sha=shas)
    dve_ops_mod.OPS.append(op)
    dve_ops_mod._SUB_OPCODE_FOR_NAME[name] = row
    dve_ops_mod.CUSTOM_DVE_SPECS[name] = spec
    return op


def _get_shrink_op():
    """out = v - clamp(v, -s1, s1) with v = in0 + s0*in1  (softshrink fused
    with the momentum-weighted x add; in0 comes straight from PSUM)."""
    from concourse.dve_spec import Spec, Src0, Src1, C0, C1, C2, maxx, minn

    v = Src0 + C0 * Src1
    body = v - minn(maxx(v, C2), C1)

    def _ref(in0, in1, s0, s1, imm2):
        v = in0.astype(np.float32) + np.float32(s0) * in1.astype(np.float32)
        return v - np.minimum(np.maximum(v, np.float32(imm2)), np.float32(s1))

    return _register_dve_op("FISTA_SHRINK_ANT", Spec(body=body, reference=_ref))


def _get_shrink0_op():
    """out = in0 - clamp(in0, -s1, s1)  (softshrink only; used at iteration 0
    where x_0 = 0 so there is no momentum term)."""
    from concourse.dve_spec import Spec, Src0, C0, C1, maxx, minn

    body = Src0 - minn(maxx(Src0, C0), C1)

    def _ref(in0, in1, s0, s1, imm2):
        v = in0.astype(np.float32)
        return v - np.minimum(np.maximum(v, np.float32(s0)), np.float32(s1))

    return _register_dve_op("FISTA_SHRINK0_ANT", Spec(body=body, reference=_ref))


# --------------------------------------------------------------------------- #
# host-side precompute
# --------------------------------------------------------------------------- #
def _build_dictionary(rr, theta, t):
    i = np.arange(t, dtype=np.float64)[:, None]
    rr = rr.astype(np.float64)
    theta = theta.astype(np.float64)
    rp = rr[None, :] ** i
    sgn = np.where(np.arange(t)[:, None] % 2 == 0, 1.0, -1.0)
    c = np.cos(i * theta[None, :])
    s = np.sin(i * theta[None, :])
    ones = np.ones((t, 1))
    dic = np.concatenate([ones, rp * c, sgn * rp * c, rp * s, sgn * rp * s], axis=1)
    g = np.linalg.norm(dic, axis=0)
    g = np.where(g == 0, np.sqrt(t), g)
    return dic / g


def _momentum_coeffs(n_iter):
    ts = []
    t = 1.0
    for _ in range(n_iter):
        t_new = (1.0 + np.sqrt(1.0 + 4.0 * t * t)) / 2.0
        ts.append((t - 1.0) / t_new)
        t = t_new
    return np.asarray(ts, dtype=np.float32)


# --------------------------------------------------------------------------- #
# device module
# --------------------------------------------------------------------------- #
def _build_module(lam, linv, tts):
    import concourse.bacc as bacc
    import concourse.mybir as mybir
    import concourse.tile as tile

    F32 = mybir.dt.float32
    F32R = mybir.dt.float32r
    shrink_op = _get_shrink_op()
    shrink0_op = _get_shrink0_op()

    nc = bacc.Bacc("TRN2", target_bir_lowering=False, debug=False)

    y_d = nc.dram_tensor("y_in", [T, P], F32R, kind="ExternalInput").ap()
    sy_d = nc.dram_tensor("s_y", [T, 42], F32R, kind="ExternalInput").ap()
    sd_d = nc.dram_tensor("s_d", [K, 42], F32R, kind="ExternalInput").ap()
    wab_d = nc.dram_tensor("w_ab", [42, 768], F32R, kind="ExternalInput").ap()
    z_d = nc.dram_tensor("zeros", [22, P], F32R, kind="ExternalInput").ap()
    i_d = nc.dram_tensor("i_const", [128, 128], F32R, kind="ExternalInput").ap()
    out_d = nc.dram_tensor("out", [K, P], F32, kind="ExternalOutput").ap()

    # per-iteration scalars (fp32-exact python floats)
    tt_prev = [0.0] + [float(tts[j]) for j in range(NITER - 1)]
    lam_f = float(np.float32(lam))
    linv_f = float(np.float32(linv))

    with tile.TileContext(nc) as tc:
        with (
            tc.tile_pool(name="const", bufs=1) as const,
            tc.tile_pool(name="state", bufs=1) as state,
            tc.tile_pool(name="iscp", bufs=2) as iscp,
            tc.tile_pool(name="upool", bufs=1, space="PSUM") as upool,
            tc.tile_pool(name="wpool", bufs=3, space="PSUM") as wpool,
        ):
            y_t = const.tile([T, P], F32R, tag="y", name="y_t")
            sy_t = const.tile([T, 42], F32R, tag="sy", name="sy_t")
            wab_t = const.tile([42, 768], F32R, tag="wab", name="wab_t")
            i_t = const.tile([128, 128], F32R, tag="ic", name="i_t")
            sd_t = [const.tile([CH[c], 42], F32R, tag=f"sd{c}", name=f"sd_t{c}") for c in range(6)]

            nc.sync.dma_start(out=y_t[:], in_=y_d[:])
            nc.sync.dma_start(out=sy_t[:], in_=sy_d[:])
            nc.sync.dma_start(out=wab_t[:], in_=wab_d[:])
            nc.sync.dma_start(out=i_t[:], in_=i_d[:])
            for c in range(6):
                nc.sync.dma_start(out=sd_t[c][:], in_=sd_d[OFF[c]:OFF[c] + CH[c], :])

            xt = [[state.tile([CH[c], P], F32R, tag=f"x{g}_{c}", name=f"x{g}_{c}") for c in range(6)]
                  for g in range(3)]
            ab_ts = [state.tile([42, P], F32R, tag=f"AB{p}", name=f"ab_t{p}")
                     for p in range(2)]
            # rows 10..31 are dead contraction lanes of the merged matmul:
            # must be finite (stationary rows there are zero)
            for p in range(2):
                nc.sync.dma_start(out=ab_ts[p][10:32, :], in_=z_d[:])

            # Iteration specialization (avoids any zero-init):
            #   j=0: x_0 = x_{-1} = 0 -> u_0 = Y (no x-stream), no identity
            #        matmul, no m2old, plain shrink (no momentum add).
            #   j=1: tt_prev = tts[0] = 0 -> no identity matmul; m2old runs
            #        with B_0 (which is itself zero since b_scale(0) = 0).
            for j in range(NITER):
                ttp = tt_prev[j]
                gm1, g0, g1 = (j + 2) % 3, j % 3, (j + 1) % 3
                ab_cur = ab_ts[j % 2]
                ab_next = ab_ts[(j + 1) % 2]
                a_scale = float(np.float32((1.0 + ttp) * linv_f))
                b_scale = float(np.float32(-float(tts[j]) * linv_f))
                has_ux = j >= 1        # x_j nonzero
                has_id = ttp != 0.0    # j >= 2
                has_m2old = j >= 1     # B_{j-1} exists

                if has_id:
                    # scaled identity for the -tt * x_{j-1} term
                    isc = iscp.tile([128, 128], F32R, tag="isc", name="isc")
                    nc.scalar.mul(isc[:], i_t[:], float(np.float32(-ttp)))

                # u = Y - D x_j  (3-replicated across partition groups 0/32/64)
                u_ps = upool.tile([42, P], F32, tag="u", name="u_ps")
                for h in (0, 1):
                    sl = slice(NH * h, NH * (h + 1))
                    nc.tensor.matmul(u_ps[:, sl], sy_t[:], y_t[:, sl],
                                     start=True, stop=not has_ux)
                    if has_ux:
                        for c in range(6):
                            nc.tensor.matmul(u_ps[:, sl], sd_t[c][:],
                                             xt[g0][c][:, sl],
                                             start=False, stop=(c == 5))
                # scaled copies: A_j = (1+tt)/L u_j (used now),
                #                B_j = -tts[j]/L u_j (used next iteration)
                for h in (0, 1):
                    sl = slice(NH * h, NH * (h + 1))
                    eng = nc.scalar if ACT_COPY else nc.vector
                    if ACT_COPY:
                        nc.scalar.mul(ab_cur[0:T, sl], u_ps[0:T, sl], a_scale)
                    else:
                        nc.vector.tensor_scalar_mul(ab_cur[0:T, sl],
                                                    u_ps[0:T, sl], a_scale)
                if j < NITER - 1:
                    for h in (0, 1):
                        sl = slice(NH * h, NH * (h + 1))
                        if ACT_COPY:
                            nc.scalar.mul(ab_next[32:42, sl],
                                          u_ps[32:42, sl], b_scale)
                        else:
                            nc.vector.tensor_scalar_mul(ab_next[32:42, sl],
                                                        u_ps[32:42, sl],
                                                        b_scale)

                for wv in (0, 1):
                    cs = [3 * wv, 3 * wv + 1, 3 * wv + 2]
                    wt = {c: wpool.tile([CH[c], P], F32, tag="w", name=f"w{c}")
                          for c in cs}
                    # identity part: w = -tt * x_{j-1}
                    if has_id:
                        for h in (0, 1):
                            sl = slice(NH * h, NH * (h + 1))
                            for c in cs:
                                nc.tensor.matmul(
                                    wt[c][:, sl],
                                    isc[0:CH[c], 0:CH[c]],
                                    xt[gm1][c][:, sl],
                                    start=True, stop=False)
                    # rank-10+10 part in one matmul: w += [D;0;D]^T [A;junk;B]
                    kc = 42 if has_m2old else T
                    for h in (0, 1):
                        sl = slice(NH * h, NH * (h + 1))
                        for c in cs:
                            nc.tensor.matmul(
                                wt[c][:, sl],
                                wab_t[0:kc, 128 * c:128 * c + CH[c]],
                                ab_cur[0:kc, sl],
                                start=not has_id, stop=True)
                    # x_{j+1} = shrink(w + (1+tt) x_j)
                    for c in cs:
                        if has_ux:
                            nc.vector._custom_dve(
                                shrink_op, out=xt[g1][c][:], in0=wt[c][:],
                                in1=xt[g0][c][:],
                                s0=float(np.float32(1.0 + ttp)), s1=lam_f,
                                imm2=-lam_f)
                        else:
                            nc.vector._custom_dve(
                                shrink0_op, out=xt[g1][c][:], in0=wt[c][:],
                                s0=-lam_f, s1=lam_f)
                        if j == NITER - 1:
                            nc.sync.dma_start(
                                out=out_d[OFF[c]:OFF[c] + CH[c], :],
                                in_=xt[g1][c][:].bitcast(F32))

    nc.compile()
    return nc


# --------------------------------------------------------------------------- #
# entry point
# --------------------------------------------------------------------------- #
def _prepare(x, Drr, Dtheta, t):
    x = np.asarray(x, dtype=np.float32)
    d64 = _build_dictionary(np.asarray(Drr), np.asarray(Dtheta), t)
    dtd = d64.T @ d64
    lspec = np.linalg.norm(dtd, ord=2)
    linv = 1.0 / lspec
    lam = LAMBD * linv
    d32 = d64.astype(np.float32)
    tts = _momentum_coeffs(NITER)

    # u = Y - D x is produced replicated at partition offsets 0 and 32 (the
    # 0-copy feeds the A scaled-copy, the 32-copy feeds the B scaled-copy).
    s_y = np.zeros((T, 42), dtype=np.float32)
    for r in (0, 1):
        s_y[np.arange(T), 32 * r + np.arange(T)] = 1.0
    s_d = np.zeros((K, 42), dtype=np.float32)
    for r in (0, 1):
        s_d[:, 32 * r:32 * r + T] = -d32.T
    # merged rank-20 stationary: rows 0..9 multiply A, rows 32..41 multiply B
    w_ab = np.zeros((42, 768), dtype=np.float32)
    for c in range(6):
        w_ab[0:T, 128 * c:128 * c + CH[c]] = d32[:, OFF[c]:OFF[c] + CH[c]]
        w_ab[32:42, 128 * c:128 * c + CH[c]] = d32[:, OFF[c]:OFF[c] + CH[c]]
    i_const = np.eye(128, dtype=np.float32)
    zeros = np.zeros((22, P), dtype=np.float32)
    return x, lam, linv, tts, s_y, s_d, w_ab, i_const, zeros


def run(x, Drr, Dtheta, T_in, trace=False):
    from concourse.bass_utils import run_bass_kernel_spmd

    t = int(np.asarray(T_in))
    assert t == T
    x, lam, linv, tts, s_y, s_d, w_ab, i_const, zeros = _prepare(x, Drr, Dtheta, t)

    key = ("mod", float(np.float32(lam)), float(np.float32(linv)))
    if key not in _cache:
        _cache[key] = _build_module(lam, linv, tts)
    nc = _cache[key]

    in_maps = []
    for core in range(N_CORES):
        in_maps.append({
            "y_in": np.ascontiguousarray(x[0, :, core * P:(core + 1) * P]),
            "s_y": s_y,
            "s_d": s_d,
            "w_ab": w_ab,
            "i_const": i_const,
            "zeros": zeros,
        })
    res = run_bass_kernel_spmd(nc, in_maps, list(range(N_CORES)), trace=trace)
    out = np.concatenate([res.results[c]["out"] for c in range(N_CORES)], axis=1)
    return out[None, :, :].astype(np.float32), res


def kernel(x, Drr, Dtheta, T, **kw):
    out, _ = run(x, Drr, Dtheta, T, trace=bool(os.environ.get("FISTA_TRACE")))
    return out



# revision 2
# speedup vs baseline: 1.0062x; 1.0062x over previous
"""DYAN encoder (FISTA sparse coding) as a Bass/Tile kernel on 8 trn2 NeuronCores.

Algorithm notes
---------------
reference computes, with D [T=10, K=645] (normalized dictionary), Y = x[0] [10, P]:
    A   = I - D^T D / L,  c = D^T Y / L,  lam = 0.1 / L
    y_0 = x_0 = 0
    for j in 0..99:   (the early-stop never triggers for this data)
        w      = A y_j + c = y_j + (1/L) D^T (Y - D y_j)
        x_{j+1} = softshrink(w, lam)
        y_{j+1} = (1+tt_j) x_{j+1} - tt_j x_j
Since A is I minus a rank-10 term, each iteration only needs thin matmuls:
    u_j = Y - D x_j                    [10, P]   (PE, contraction 645)
    r_j = (1+tt) u_j - tt u_{j-1}      (momentum folded into the residual)
    w   = (1/L) D^T r_j - tt x_{j-1} + (1+tt) x_j + ... (identity parts)
    x_{j+1} = shrink((1/L) D^T r_j - tt x_{j-1}  +  (1+tt) x_j)
The (1/L)(1+tt) / -(1/L)tt scalings ride the PSUM->SBUF copies of u (ScalarE),
the -tt x_{j-1} term is a scaled-identity matmul on PE, and the (1+tt) x_j add
plus softshrink is one fused custom DVE op per chunk.

Sharding: pure data parallel over the pixel dim P (8192 -> 8 x 1024).
"""

import os
import numpy as np

T = 10
NDICT = 161
K = 4 * NDICT + 1          # 645
P_FULL = 8192
N_CORES = 8
P = P_FULL // N_CORES      # 1024
NH = 512                   # psum-bank half width (fp32)
CH = [128, 128, 128, 128, 128, 5]   # K split into partition chunks
OFF = [0, 128, 256, 384, 512, 640]
NITER = 100
LAMBD = 0.1

# debug/ablation flags
ACT_COPY = True      # A/B copies on ScalarE (else VectorE)

_cache = {}


# --------------------------------------------------------------------------- #
# custom DVE ops
# --------------------------------------------------------------------------- #
def _register_dve_op(name, spec):
    import concourse.dve_ops as dve_ops_mod
    from concourse.dve_spec import lower, _has_src1
    from concourse.dve_uop import DveOpSpec

    for o in dve_ops_mod.OPS:
        if o.name == name:
            return o
    row = dve_ops_mod._CUSTOM_DVE_ROW_BASE + len(dve_ops_mod.OPS)
    assert row < 0x20, "DVE opcode rows exhausted"
    shas = {}
    for ver in ("v3", "v4"):
        s = DveOpSpec(name=name, opcode=row, uops=lower(spec, ver=ver),
                      rd1_en=_has_src1(spec))
        shas[ver] = s.sha(ver)
    op = dve_ops_mod.DveOp(name, spec, subdim=False, uops_sha=shas)
    dve_ops_mod.OPS.append(op)
    dve_ops_mod._SUB_OPCODE_FOR_NAME[name] = row
    dve_ops_mod.CUSTOM_DVE_SPECS[name] = spec
    return op


def _get_shrink_op():
    """out = v - clamp(v, -s1, s1) with v = in0 + s0*in1  (softshrink fused
    with the momentum-weighted x add; in0 comes straight from PSUM)."""
    from concourse.dve_spec import Spec, Src0, Src1, C0, C1, C2, maxx, minn

    v = Src0 + C0 * Src1
    body = v - minn(maxx(v, C2), C1)

    def _ref(in0, in1, s0, s1, imm2):
        v = in0.astype(np.float32) + np.float32(s0) * in1.astype(np.float32)
        return v - np.minimum(np.maximum(v, np.float32(imm2)), np.float32(s1))

    return _register_dve_op("FISTA_SHRINK_ANT", Spec(body=body, reference=_ref))


def _get_shrink0_op():
    """out = in0 - clamp(in0, -s1, s1)  (softshrink only; used at iteration 0
    where x_0 = 0 so there is no momentum term)."""
    from concourse.dve_spec import Spec, Src0, C0, C1, maxx, minn

    body = Src0 - minn(maxx(Src0, C0), C1)

    def _ref(in0, in1, s0, s1, imm2):
        v = in0.astype(np.float32)
        return v - np.minimum(np.maximum(v, np.float32(s0)), np.float32(s1))

    return _register_dve_op("FISTA_SHRINK0_ANT", Spec(body=body, reference=_ref))


# --------------------------------------------------------------------------- #
# host-side precompute
# --------------------------------------------------------------------------- #
def _build_dictionary(rr, theta, t):
    i = np.arange(t, dtype=np.float64)[:, None]
    rr = rr.astype(np.float64)
    theta = theta.astype(np.float64)
    rp = rr[None, :] ** i
    sgn = np.where(np.arange(t)[:, None] % 2 == 0, 1.0, -1.0)
    c = np.cos(i * theta[None, :])
    s = np.sin(i * theta[None, :])
    ones = np.ones((t, 1))
    dic = np.concatenate([ones, rp * c, sgn * rp * c, rp * s, sgn * rp * s], axis=1)
    g = np.linalg.norm(dic, axis=0)
    g = np.where(g == 0, np.sqrt(t), g)
    return dic / g


def _momentum_coeffs(n_iter):
    ts = []
    t = 1.0
    for _ in range(n_iter):
        t_new = (1.0 + np.sqrt(1.0 + 4.0 * t * t)) / 2.0
        ts.append((t - 1.0) / t_new)
        t = t_new
    return np.asarray(ts, dtype=np.float32)


# --------------------------------------------------------------------------- #
# device module
# --------------------------------------------------------------------------- #
def _build_module(lam, linv, tts):
    import concourse.bacc as bacc
    import concourse.mybir as mybir
    import concourse.tile as tile

    F32 = mybir.dt.float32
    F32R = mybir.dt.float32r
    shrink_op = _get_shrink_op()
    shrink0_op = _get_shrink0_op()

    nc = bacc.Bacc("TRN2", target_bir_lowering=False, debug=False)

    y_d = nc.dram_tensor("y_in", [T, P], F32R, kind="ExternalInput").ap()
    sy_d = nc.dram_tensor("s_y", [T, 42], F32R, kind="ExternalInput").ap()
    sd_d = nc.dram_tensor("s_d", [K, 42], F32R, kind="ExternalInput").ap()
    wab_d = nc.dram_tensor("w_ab", [42, 768], F32R, kind="ExternalInput").ap()
    z_d = nc.dram_tensor("zeros", [22, P], F32R, kind="ExternalInput").ap()
    i_d = nc.dram_tensor("i_const", [128, 128], F32R, kind="ExternalInput").ap()
    out_d = nc.dram_tensor("out", [K, P], F32, kind="ExternalOutput").ap()

    # per-iteration scalars (fp32-exact python floats)
    tt_prev = [0.0] + [float(tts[j]) for j in range(NITER - 1)]
    lam_f = float(np.float32(lam))
    linv_f = float(np.float32(linv))

    with tile.TileContext(nc) as tc:
        with (
            tc.tile_pool(name="const", bufs=1) as const,
            tc.tile_pool(name="state", bufs=1) as state,
            tc.tile_pool(name="iscp", bufs=2) as iscp,
            tc.tile_pool(name="upool", bufs=1, space="PSUM") as upool,
            tc.tile_pool(name="wpool", bufs=3, space="PSUM") as wpool,
        ):
            y_t = const.tile([T, P], F32R, tag="y", name="y_t")
            sy_t = const.tile([T, 42], F32R, tag="sy", name="sy_t")
            wab_t = const.tile([42, 768], F32R, tag="wab", name="wab_t")
            i_t = const.tile([128, 128], F32R, tag="ic", name="i_t")
            sd_t = [const.tile([CH[c], 42], F32R, tag=f"sd{c}", name=f"sd_t{c}") for c in range(6)]

            nc.sync.dma_start(out=y_t[:], in_=y_d[:])
            nc.sync.dma_start(out=sy_t[:], in_=sy_d[:])
            nc.sync.dma_start(out=wab_t[:], in_=wab_d[:])
            nc.sync.dma_start(out=i_t[:], in_=i_d[:])
            for c in range(6):
                nc.sync.dma_start(out=sd_t[c][:], in_=sd_d[OFF[c]:OFF[c] + CH[c], :])

            xt = [[state.tile([CH[c], P], F32R, tag=f"x{g}_{c}", name=f"x{g}_{c}") for c in range(6)]
                  for g in range(3)]
            ab_ts = [state.tile([42, P], F32R, tag=f"AB{p}", name=f"ab_t{p}")
                     for p in range(2)]
            # rows 10..31 are dead contraction lanes of the merged matmul:
            # must be finite (stationary rows there are zero)
            for p in range(2):
                nc.sync.dma_start(out=ab_ts[p][10:32, :], in_=z_d[:])

            # Iteration specialization (avoids any zero-init):
            #   j=0: x_0 = x_{-1} = 0 -> u_0 = Y (no x-stream), no identity
            #        matmul, no m2old, plain shrink (no momentum add).
            #   j=1: tt_prev = tts[0] = 0 -> no identity matmul; m2old runs
            #        with B_0 (which is itself zero since b_scale(0) = 0).
            for j in range(NITER):
                ttp = tt_prev[j]
                gm1, g0, g1 = (j + 2) % 3, j % 3, (j + 1) % 3
                ab_cur = ab_ts[j % 2]
                ab_next = ab_ts[(j + 1) % 2]
                a_scale = float(np.float32((1.0 + ttp) * linv_f))
                b_scale = float(np.float32(-float(tts[j]) * linv_f))
                has_ux = j >= 1        # x_j nonzero
                has_id = ttp != 0.0    # j >= 2
                has_m2old = j >= 1     # B_{j-1} exists

                if has_id:
                    # scaled identity for the -tt * x_{j-1} term
                    isc = iscp.tile([128, 128], F32R, tag="isc", name="isc")
                    nc.scalar.mul(isc[:], i_t[:], float(np.float32(-ttp)))

                # u = Y - D x_j, replicated at partition groups 0/32.
                # Per-half PSUM tiles (1 bank each) so the WAR against this
                # iteration's A/B copies clears per half, not per iteration —
                # a whole-tile WAR stalled the next u-matmul ~457ns once per
                # 2 iterations, and each stall cost a 27us HAM re-throttle.
                for h in (0, 1):
                    sl = slice(NH * h, NH * (h + 1))
                    u_ps = upool.tile([42, NH], F32, tag=f"u{h}", name=f"u_ps{h}")
                    nc.tensor.matmul(u_ps[:], sy_t[:], y_t[:, sl],
                                     start=True, stop=not has_ux)
                    if has_ux:
                        for c in range(6):
                            nc.tensor.matmul(u_ps[:], sd_t[c][:],
                                             xt[g0][c][:, sl],
                                             start=False, stop=(c == 5))
                    # scaled copies: A_j = (1+tt)/L u_j (used now),
                    #                B_j = -tts[j]/L u_j (used next iteration)
                    nc.scalar.mul(ab_cur[0:T, sl], u_ps[0:T, :], a_scale)
                    if j < NITER - 1:
                        nc.scalar.mul(ab_next[32:42, sl],
                                      u_ps[32:42, :], b_scale)

                for wv in (0, 1):
                    cs = [3 * wv, 3 * wv + 1, 3 * wv + 2]
                    wt = {c: wpool.tile([CH[c], P], F32, tag="w", name=f"w{c}")
                          for c in cs}
                    # identity part: w = -tt * x_{j-1}
                    if has_id:
                        for h in (0, 1):
                            sl = slice(NH * h, NH * (h + 1))
                            for c in cs:
                                nc.tensor.matmul(
                                    wt[c][:, sl],
                                    isc[0:CH[c], 0:CH[c]],
                                    xt[gm1][c][:, sl],
                                    start=True, stop=False)
                    # rank-10+10 part in one matmul: w += [D;0;D]^T [A;junk;B]
                    kc = 42 if has_m2old else T
                    for h in (0, 1):
                        sl = slice(NH * h, NH * (h + 1))
                        for c in cs:
                            nc.tensor.matmul(
                                wt[c][:, sl],
                                wab_t[0:kc, 128 * c:128 * c + CH[c]],
                                ab_cur[0:kc, sl],
                                start=not has_id, stop=True)
                    # x_{j+1} = shrink(w + (1+tt) x_j)
                    for c in cs:
                        if has_ux:
                            nc.vector._custom_dve(
                                shrink_op, out=xt[g1][c][:], in0=wt[c][:],
                                in1=xt[g0][c][:],
                                s0=float(np.float32(1.0 + ttp)), s1=lam_f,
                                imm2=-lam_f)
                        else:
                            nc.vector._custom_dve(
                                shrink0_op, out=xt[g1][c][:], in0=wt[c][:],
                                s0=-lam_f, s1=lam_f)
                        if j == NITER - 1:
                            nc.sync.dma_start(
                                out=out_d[OFF[c]:OFF[c] + CH[c], :],
                                in_=xt[g1][c][:].bitcast(F32))

    nc.compile()
    return nc


# --------------------------------------------------------------------------- #
# entry point
# --------------------------------------------------------------------------- #
def _prepare(x, Drr, Dtheta, t):
    x = np.asarray(x, dtype=np.float32)
    d64 = _build_dictionary(np.asarray(Drr), np.asarray(Dtheta), t)
    dtd = d64.T @ d64
    lspec = np.linalg.norm(dtd, ord=2)
    linv = 1.0 / lspec
    lam = LAMBD * linv
    d32 = d64.astype(np.float32)
    tts = _momentum_coeffs(NITER)

    # u = Y - D x is produced replicated at partition offsets 0 and 32 (the
    # 0-copy feeds the A scaled-copy, the 32-copy feeds the B scaled-copy).
    s_y = np.zeros((T, 42), dtype=np.float32)
    for r in (0, 1):
        s_y[np.arange(T), 32 * r + np.arange(T)] = 1.0
    s_d = np.zeros((K, 42), dtype=np.float32)
    for r in (0, 1):
        s_d[:, 32 * r:32 * r + T] = -d32.T
    # merged rank-20 stationary: rows 0..9 multiply A, rows 32..41 multiply B
    w_ab = np.zeros((42, 768), dtype=np.float32)
    for c in range(6):
        w_ab[0:T, 128 * c:128 * c + CH[c]] = d32[:, OFF[c]:OFF[c] + CH[c]]
        w_ab[32:42, 128 * c:128 * c + CH[c]] = d32[:, OFF[c]:OFF[c] + CH[c]]
    i_const = np.eye(128, dtype=np.float32)
    zeros = np.zeros((22, P), dtype=np.float32)
    return x, lam, linv, tts, s_y, s_d, w_ab, i_const, zeros


def run(x, Drr, Dtheta, T_in, trace=False):
    from concourse.bass_utils import run_bass_kernel_spmd

    t = int(np.asarray(T_in))
    assert t == T
    x, lam, linv, tts, s_y, s_d, w_ab, i_const, zeros = _prepare(x, Drr, Dtheta, t)

    key = ("mod", float(np.float32(lam)), float(np.float32(linv)))
    if key not in _cache:
        _cache[key] = _build_module(lam, linv, tts)
    nc = _cache[key]

    in_maps = []
    for core in range(N_CORES):
        in_maps.append({
            "y_in": np.ascontiguousarray(x[0, :, core * P:(core + 1) * P]),
            "s_y": s_y,
            "s_d": s_d,
            "w_ab": w_ab,
            "i_const": i_const,
            "zeros": zeros,
        })
    res = run_bass_kernel_spmd(nc, in_maps, list(range(N_CORES)), trace=trace)
    out = np.concatenate([res.results[c]["out"] for c in range(N_CORES)], axis=1)
    return out[None, :, :].astype(np.float32), res


def kernel(x, Drr, Dtheta, T, **kw):
    out, _ = run(x, Drr, Dtheta, T, trace=bool(os.environ.get("FISTA_TRACE")))
    return out

